# revision 16
# baseline (speedup 1.0000x reference)
"""BitNet b1.58 ternary-quantized linear on 8 Trainium2 NeuronCores.

Reference computation (single device):
    scale = clip(mean(|W|), 1e-5, 1000)
    q     = ternarize(W / scale, threshold=2/3)  in {-1, 0, +1}
    out   = x @ (q * scale).T + bias             x:[4,2048,4096] W:[4096,4096]

Sharding (2D grid over 8 cores): 4 row-groups of x (M=2048 each) x 2
feature-groups of W (N=2048 each). Shards are laid out K-major on the
host so the TensorEngine operands need no on-device transpose:
  - x shard is passed as xT [K=4096, M=2048] f32
  - W shard is passed as wt5 [16 nb, 128 ki, 32 kb, 128 n] f32 so each
    128-column n-block DMAs as contiguous 8KB-per-partition runs
  - a distinct 1/8 column-slice of W.T feeds the global |W| mean

Two launches (cheaper than a 512B AllReduce, which measures ~165us on
the ncfw path):
  A. each core reduces sum(|W|-C) over its 1/8 W slice to one scalar
     (C=f32(0.79788456)=E|N(0,1)| keeps partial sums near zero so fp32
     accumulation error stays ~1e-10 relative; C is added back on
     device in launch B). The host only concatenates the 8 scalars.
  B. main kernel: scale/threshold columns from the 8 partials, x cast
     f32->bf16 (ACT) into resident [K, M] tiles, per-n-block ternarize
     (DVE is_gt/is_lt masks -> q bf16 already [K, N]), 128x128x512
     matmuls accumulating K=4096 into PSUM, fused out = psum*scale +
     bias eviction on ACT, transposed out-shard DMA.
"""

import os

import numpy as np

import concourse.bass as bass
import concourse.tile as tile
from concourse import bacc, mybir
from concourse.bass_utils import run_bass_kernel_spmd

N_CORES = 8
R_GRP, F_GRP = 4, 2            # row groups (x) x feature groups (W)
B, S, K = 4, 2048, 4096        # x: [B, S, K]
N_OUT = 4096                   # W: [N_OUT, K]
M_ALL = B * S                  # 8192 rows of x
M_SH = M_ALL // R_GRP          # 2048 rows per core
N_SH = N_OUT // F_GRP          # 2048 out-features per core
WRED = N_OUT // N_CORES        # 512 rows of W per core for the scale reduce
KO = K // 128                  # 32 k-blocks
M_CHUNK = 512                  # matmul moving free dim
N_MC = M_SH // M_CHUNK         # 4 m-chunks
N_NB = N_SH // 128             # 16 n-blocks

C_ABS = float(np.float32(0.79788456))   # E|N(0,1)|; exact f32 constant
THRESH = 2.0 / 3.0
F32 = mybir.dt.float32
BF16 = mybir.dt.bfloat16

_CACHE = {}
LAST_RESULTS = None


def _build_scale():
    """Launch A: partial = sum(|W slice|) - n*C reduced to one scalar."""
    nc = bacc.Bacc(None, target_bir_lowering=False, num_devices=N_CORES)
    wred_d = nc.dram_tensor("wredN", [WRED, K], F32, kind="ExternalInput")
    part_d = nc.dram_tensor("partial", [1, 1], F32, kind="ExternalOutput")

    with tile.TileContext(nc) as tc:
        with (
            tc.tile_pool(name="misc", bufs=1) as misc,
            tc.tile_pool(name="redstage", bufs=4) as redstage,
            tc.tile_pool(name="psum_s", bufs=1, space="PSUM") as psum_s_pool,
        ):
            racc = misc.tile([128, 4], F32)
            for t in range(4):
                wf = redstage.tile([128, K], F32, tag="redstage")
                nc.sync.dma_start(wf[:], wred_d[128 * t:128 * (t + 1), :])
                nc.vector.tensor_reduce(
                    racc[:, t:t + 1], wf[:],
                    axis=mybir.AxisListType.X, op=mybir.AluOpType.add,
                    apply_absolute_value=True)
            # subtract the expected chunk sum K*C (fp32-exact: K is 2^12) so
            # the remaining accumulation runs on near-zero values
            rsm = misc.tile([128, 4], F32)
            nc.vector.tensor_scalar(
                rsm[:], racc[:], -float(np.float32(K * np.float32(C_ABS))), None,
                mybir.AluOpType.add)
            r1 = misc.tile([128, 1], F32)
            nc.vector.tensor_reduce(
                r1[:], rsm[:], axis=mybir.AxisListType.X, op=mybir.AluOpType.add)
            ones_col = misc.tile([128, 1], F32)
            nc.vector.memset(ones_col[:], 1.0)
            ps1 = psum_s_pool.tile([1, 1], F32)
            nc.tensor.matmul(ps1[:], lhsT=r1[:], rhs=ones_col[:])
            sc = misc.tile([1, 1], F32)
            nc.vector.tensor_copy(sc[:], ps1[:])
            nc.sync.dma_start(part_d[:], sc[:])

    nc.compile()
    return nc


def _build_main():
    nc = bacc.Bacc(None, target_bir_lowering=False, num_devices=N_CORES)
    xt_d = nc.dram_tensor("xt_sh", [K, M_SH], F32, kind="ExternalInput")
    wt5_d = nc.dram_tensor("wt5", [N_NB, 128, KO, 128], F32, kind="ExternalInput")
    part_d = nc.dram_tensor("partials", [N_CORES], F32, kind="ExternalInput")
    bias_d = nc.dram_tensor("bias_sh", [N_SH], F32, kind="ExternalInput")
    outT = nc.dram_tensor("outT", [N_SH, M_SH], F32, kind="ExternalOutput")

    with tile.TileContext(nc) as tc:
        with (
            tc.tile_pool(name="misc", bufs=1) as misc,
            tc.tile_pool(name="xstage", bufs=4) as xstage,
            tc.tile_pool(name="wq", bufs=2) as wq_pool,
            tc.tile_pool(name="masks", bufs=2) as mask_pool,
            tc.tile_pool(name="qt", bufs=3) as qt_pool,
            tc.tile_pool(name="outp", bufs=2) as out_pool,
            tc.tile_pool(name="psum", bufs=4, space="PSUM") as psum_pool,
            tc.tile_pool(name="psum_s", bufs=1, space="PSUM") as psum_s_pool,
        ):
            # ---- scale / threshold columns from the 8 raw partials
            pt = misc.tile([1, N_CORES], F32)
            nc.sync.dma_start(pt[:], part_d.rearrange("(p o) -> p o", p=1))
            s0 = misc.tile([1, 1], F32)
            nc.vector.tensor_reduce(
                s0[:], pt[:], axis=mybir.AxisListType.X, op=mybir.AluOpType.add)
            ones_row = misc.tile([1, 128], F32)
            nc.vector.memset(ones_row[:], 1.0)
            ps_bc = psum_s_pool.tile([128, 1], F32)
            nc.tensor.matmul(ps_bc[:], lhsT=ones_row[:], rhs=s0[:])
            mean_col = misc.tile([128, 1], F32)
            nc.vector.tensor_scalar(
                mean_col[:], ps_bc[:], 1.0 / (N_OUT * K), C_ABS,
                mybir.AluOpType.mult, mybir.AluOpType.add)
            s_col = misc.tile([128, 1], F32)
            nc.vector.tensor_scalar(
                s_col[:], mean_col[:], 1e-5, 1000.0,
                mybir.AluOpType.max, mybir.AluOpType.min)
            thr_col = misc.tile([128, 1], F32)
            nc.vector.tensor_scalar(
                thr_col[:], s_col[:], THRESH, None, mybir.AluOpType.mult)
            nthr_col = misc.tile([128, 1], F32)
            nc.vector.tensor_scalar(
                nthr_col[:], s_col[:], -THRESH, None, mybir.AluOpType.mult)

            # bias (per out-feature) laid out [partition=n%128, col=n//128]
            bias_sb = misc.tile([128, N_NB], F32)
            nc.sync.dma_start(bias_sb[:], bias_d.rearrange("(o p) -> p o", p=128))

            def emit_quant(nb, qt, pfx=""):
                for h in range(2):
                    wq = wq_pool.tile([128, KO // 2, 128], F32, tag="wq",
                                      name=f"wq{pfx}{nb}_{h}")
                    (nc.sync if h == 0 else nc.scalar).dma_start(
                        wq[:], wt5_d[nb, :, 16 * h:16 * (h + 1), :])
                    wq_f = wq[:].rearrange("p a b -> p (a b)")
                    mpos = mask_pool.tile([128, 2048], BF16, tag="masks",
                                          name=f"mp{pfx}{nb}_{h}")
                    nc.vector.tensor_scalar(
                        mpos[:], wq_f, thr_col[:], None, mybir.AluOpType.is_gt)
                    mneg = mask_pool.tile([128, 2048], BF16, tag="masks",
                                          name=f"mn{pfx}{nb}_{h}")
                    nc.vector.tensor_scalar(
                        mneg[:], wq_f, nthr_col[:], None, mybir.AluOpType.is_lt)
                    nc.vector.tensor_tensor(
                        qt[:, 16 * h:16 * (h + 1), :].rearrange("p a b -> p (a b)"),
                        mpos[:], mneg[:], mybir.AluOpType.subtract)

            # quant for nb=0 first so its DMA precedes the x stream
            qts = [qt_pool.tile([128, KO, 128], BF16, tag="qt", name=f"qt{nb}")
                   for nb in range(2)]
            emit_quant(0, qts[0])

            # ---- x -> bf16 resident [128ki, 512m] tiles, one per (mc, kb)
            # so each matmul depends only on its own k-block's cast, and the
            # accumulation chains pipeline right behind the x DMA stream.
            # Loads alternate between the two HWDGE rings (sync + scalar).
            xt = [[misc.tile([128, M_CHUNK], BF16, name=f"xt{mc}_{kb}")
                   for kb in range(KO)] for mc in range(N_MC)]
            for mc in range(N_MC):
                for kb in range(KO):
                    xf = xstage.tile([128, M_CHUNK], F32, tag="xstage")
                    dma_eng = nc.sync if kb % 2 == 0 else nc.scalar
                    dma_eng.dma_start(
                        xf[:], xt_d[128 * kb:128 * (kb + 1),
                                    M_CHUNK * mc:M_CHUNK * (mc + 1)])
                    # split the f32->bf16 casts across ACT and DVE
                    if kb % 2 == 0:
                        nc.scalar.copy(xt[mc][kb][:], xf[:])
                    else:
                        nc.vector.tensor_copy(xt[mc][kb][:], xf[:])

            emit_quant(1, qts[1])

            def chain(nb, mc, qt):
                ps = psum_pool.tile([128, M_CHUNK], F32, tag="psum",
                                    name=f"ps{nb}_{mc}")
                for ko in range(KO):
                    nc.tensor.matmul(
                        ps[:], lhsT=qt[:, ko, :],
                        rhs=xt[mc][ko][:],
                        start=(ko == 0), stop=(ko == KO - 1))
                ob = out_pool.tile([128, M_CHUNK], F32, tag="outp",
                                   name=f"ob{nb}_{mc}")
                nc.scalar.activation(
                    ob[:], ps[:], mybir.ActivationFunctionType.Identity,
                    bias=bias_sb[:, nb:nb + 1], scale=s_col[:])
                nc.scalar.dma_start(
                    outT[128 * nb:128 * (nb + 1),
                         M_CHUNK * mc:M_CHUNK * (mc + 1)], ob[:])

            # ---- per n-block: matmul + fused evict (quant pipelined 2 ahead).
            # The first DEFER n-blocks only run m-chunks 0/1 (x still
            # streaming); their mc 2/3 chains run in a tail pass with a cheap
            # re-quantize, so no chain ever stalls holding a PSUM slot.
            DEFER = 0
            for nb in range(N_NB):
                qt = qts[nb]
                if nb + 2 < N_NB:
                    qts.append(qt_pool.tile([128, KO, 128], BF16, tag="qt",
                                            name=f"qt{nb + 2}"))
                    emit_quant(nb + 2, qts[nb + 2])
                mcs = (0, 1) if nb < DEFER else (0, 1, 2, 3)
                for mc in mcs:
                    chain(nb, mc, qt)
            for nb in range(DEFER):
                qt2 = qt_pool.tile([128, KO, 128], BF16, tag="qt",
                                   name=f"qt2_{nb}")
                emit_quant(nb, qt2, pfx="t")
                for mc in (2, 3):
                    chain(nb, mc, qt2)

    nc.compile()
    return nc


def kernel(x, weight, bias):
    global LAST_RESULTS
    x = np.asarray(x, dtype=np.float32)
    weight = np.ascontiguousarray(np.asarray(weight, dtype=np.float32))
    bias = np.ascontiguousarray(np.asarray(bias, dtype=np.float32))

    if "nc_scale" not in _CACHE:
        _CACHE["nc_scale"] = _build_scale()
        _CACHE["nc_main"] = _build_main()
    nc_scale, nc_main = _CACHE["nc_scale"], _CACHE["nc_main"]

    trace = bool(int(os.environ.get("KERNEL_TRACE", "0")))
    kw = {"trace": True, "trace_cores": [0]} if trace else {}

    # Launch A: distributed |W| partial sums (one distinct 1/8 slice each)
    in_a = [{"wredN": weight[WRED * c:WRED * (c + 1)]}
            for c in range(N_CORES)]
    res_a = run_bass_kernel_spmd(nc_scale, in_a, list(range(N_CORES)), **kw)
    partials = np.array(
        [res_a.results[c]["partial"][0, 0] for c in range(N_CORES)],
        dtype=np.float32)

    # Launch B: the matmul kernel
    xr = x.reshape(M_ALL, K)
    in_b = []
    for c in range(N_CORES):
        i, j = c // F_GRP, c % F_GRP
        w_sh = weight[N_SH * j:N_SH * (j + 1)]          # [2048 n, 4096 k]
        # wt5[nb, ki, kb, n] = w_sh[128*nb + n, 128*kb + ki]
        wt5 = np.ascontiguousarray(
            w_sh.reshape(N_NB, 128, KO, 128).transpose(0, 3, 2, 1))
        in_b.append({
            "xt_sh": np.ascontiguousarray(xr[M_SH * i:M_SH * (i + 1)].T),
            "wt5": wt5,
            "partials": partials,
            "bias_sh": bias[N_SH * j:N_SH * (j + 1)],
        })
    res_b = run_bass_kernel_spmd(nc_main, in_b, list(range(N_CORES)), **kw)
    LAST_RESULTS = (res_a, res_b)

    out = np.empty((M_ALL, N_OUT), dtype=np.float32)
    for c in range(N_CORES):
        i, j = c // F_GRP, c % F_GRP
        out[M_SH * i:M_SH * (i + 1), N_SH * j:N_SH * (j + 1)] = \
            res_b.results[c]["outT"].T
    return out.reshape(B, S, N_OUT)


# revision 17
# speedup vs baseline: 1.0813x; 1.0813x over previous
"""BitNet b1.58 ternary-quantized linear on 8 Trainium2 NeuronCores.

Reference computation (single device):
    scale = clip(mean(|W|), 1e-5, 1000)
    q     = ternarize(W / scale, threshold=2/3)  in {-1, 0, +1}
    out   = x @ (q * scale).T + bias             x:[4,2048,4096] W:[4096,4096]

Sharding (2D grid over 8 cores): 4 row-groups of x (M=2048 each) x 2
feature-groups of W (N=2048 each). Shards are laid out K-major on the
host so the TensorEngine operands need no on-device transpose:
  - x shard is passed as xT [K=4096, M=2048] f32
  - W shard is passed as wt5 [16 nb, 128 ki, 32 kb, 128 n] f32 so each
    128-column n-block DMAs as contiguous 8KB-per-partition runs
  - a distinct 1/8 column-slice of W.T feeds the global |W| mean

Two launches (cheaper than a 512B AllReduce, which measures ~165us on
the ncfw path):
  A. each core reduces sum(|W|-C) over its 1/8 W slice to one scalar
     (C=f32(0.79788456)=E|N(0,1)| keeps partial sums near zero so fp32
     accumulation error stays ~1e-10 relative; C is added back on
     device in launch B). The host only concatenates the 8 scalars.
  B. main kernel: scale/threshold columns from the 8 partials, x cast
     f32->bf16 (ACT) into resident [K, M] tiles, per-n-block ternarize
     (DVE is_gt/is_lt masks -> q bf16 already [K, N]), 128x128x512
     matmuls accumulating K=4096 into PSUM, fused out = psum*scale +
     bias eviction on ACT, transposed out-shard DMA.
"""

import os

import numpy as np

import concourse.bass as bass
import concourse.tile as tile
from concourse import bacc, mybir
from concourse.bass_utils import run_bass_kernel_spmd

N_CORES = 8
R_GRP, F_GRP = 4, 2            # row groups (x) x feature groups (W)
B, S, K = 4, 2048, 4096        # x: [B, S, K]
N_OUT = 4096                   # W: [N_OUT, K]
M_ALL = B * S                  # 8192 rows of x
M_SH = M_ALL // R_GRP          # 2048 rows per core
N_SH = N_OUT // F_GRP          # 2048 out-features per core
WRED = N_OUT // N_CORES        # 512 rows of W per core for the scale reduce
KO = K // 128                  # 32 k-blocks
M_CHUNK = 512                  # matmul moving free dim
N_MC = M_SH // M_CHUNK         # 4 m-chunks
N_NB = N_SH // 128             # 16 n-blocks

C_ABS = float(np.float32(0.79788456))   # E|N(0,1)|; exact f32 constant
THRESH = 2.0 / 3.0
F32 = mybir.dt.float32
BF16 = mybir.dt.bfloat16

_CACHE = {}
LAST_RESULTS = None


def _build_scale():
    """Launch A: partial = sum(|W slice|) - n*C reduced to one scalar."""
    nc = bacc.Bacc(None, target_bir_lowering=False, num_devices=N_CORES)
    wred_d = nc.dram_tensor("wredN", [WRED, K], F32, kind="ExternalInput")
    part_d = nc.dram_tensor("partial", [1, 1], F32, kind="ExternalOutput")

    with tile.TileContext(nc) as tc:
        with (
            tc.tile_pool(name="misc", bufs=1) as misc,
            tc.tile_pool(name="redstage", bufs=4) as redstage,
            tc.tile_pool(name="psum_s", bufs=1, space="PSUM") as psum_s_pool,
        ):
            racc = misc.tile([128, 4], F32)
            for t in range(4):
                wf = redstage.tile([128, K], F32, tag="redstage")
                nc.sync.dma_start(wf[:], wred_d[128 * t:128 * (t + 1), :])
                nc.vector.tensor_reduce(
                    racc[:, t:t + 1], wf[:],
                    axis=mybir.AxisListType.X, op=mybir.AluOpType.add,
                    apply_absolute_value=True)
            # subtract the expected chunk sum K*C (fp32-exact: K is 2^12) so
            # the remaining accumulation runs on near-zero values
            rsm = misc.tile([128, 4], F32)
            nc.vector.tensor_scalar(
                rsm[:], racc[:], -float(np.float32(K * np.float32(C_ABS))), None,
                mybir.AluOpType.add)
            r1 = misc.tile([128, 1], F32)
            nc.vector.tensor_reduce(
                r1[:], rsm[:], axis=mybir.AxisListType.X, op=mybir.AluOpType.add)
            ones_col = misc.tile([128, 1], F32)
            nc.vector.memset(ones_col[:], 1.0)
            ps1 = psum_s_pool.tile([1, 1], F32)
            nc.tensor.matmul(ps1[:], lhsT=r1[:], rhs=ones_col[:])
            sc = misc.tile([1, 1], F32)
            nc.vector.tensor_copy(sc[:], ps1[:])
            nc.sync.dma_start(part_d[:], sc[:])

    nc.compile()
    return nc


def _build_main():
    nc = bacc.Bacc(None, target_bir_lowering=False, num_devices=N_CORES)
    xt_d = nc.dram_tensor("xt_sh", [K, M_SH], F32, kind="ExternalInput")
    wt5_d = nc.dram_tensor("wt5", [N_NB, 128, KO, 128], F32, kind="ExternalInput")
    part_d = nc.dram_tensor("partials", [N_CORES], F32, kind="ExternalInput")
    bias_d = nc.dram_tensor("bias_sh", [N_SH], F32, kind="ExternalInput")
    outT = nc.dram_tensor("outT", [N_SH, M_SH], F32, kind="ExternalOutput")

    with tile.TileContext(nc) as tc:
        with (
            tc.tile_pool(name="misc", bufs=1) as misc,
            tc.tile_pool(name="xstage", bufs=2) as xstage,
            tc.tile_pool(name="wq", bufs=2) as wq_pool,
            tc.tile_pool(name="masks", bufs=2) as mask_pool,
            tc.tile_pool(name="qt", bufs=3) as qt_pool,
            tc.tile_pool(name="outp", bufs=2) as out_pool,
            tc.tile_pool(name="psum", bufs=4, space="PSUM") as psum_pool,
            tc.tile_pool(name="psum_s", bufs=1, space="PSUM") as psum_s_pool,
        ):
            # ---- scale / threshold columns from the 8 raw partials
            pt = misc.tile([1, N_CORES], F32)
            nc.sync.dma_start(pt[:], part_d.rearrange("(p o) -> p o", p=1))
            s0 = misc.tile([1, 1], F32)
            nc.vector.tensor_reduce(
                s0[:], pt[:], axis=mybir.AxisListType.X, op=mybir.AluOpType.add)
            ones_row = misc.tile([1, 128], F32)
            nc.vector.memset(ones_row[:], 1.0)
            ps_bc = psum_s_pool.tile([128, 1], F32)
            nc.tensor.matmul(ps_bc[:], lhsT=ones_row[:], rhs=s0[:])
            mean_col = misc.tile([128, 1], F32)
            nc.vector.tensor_scalar(
                mean_col[:], ps_bc[:], 1.0 / (N_OUT * K), C_ABS,
                mybir.AluOpType.mult, mybir.AluOpType.add)
            s_col = misc.tile([128, 1], F32)
            nc.vector.tensor_scalar(
                s_col[:], mean_col[:], 1e-5, 1000.0,
                mybir.AluOpType.max, mybir.AluOpType.min)
            thr_col = misc.tile([128, 1], F32)
            nc.vector.tensor_scalar(
                thr_col[:], s_col[:], THRESH, None, mybir.AluOpType.mult)
            nthr_col = misc.tile([128, 1], F32)
            nc.vector.tensor_scalar(
                nthr_col[:], s_col[:], -THRESH, None, mybir.AluOpType.mult)

            # bias (per out-feature) laid out [partition=n%128, col=n//128]
            bias_sb = misc.tile([128, N_NB], F32)
            nc.sync.dma_start(bias_sb[:], bias_d.rearrange("(o p) -> p o", p=128))

            def emit_quant(nb, qt, pfx=""):
                for h in range(2):
                    wq = wq_pool.tile([128, KO // 2, 128], F32, tag="wq",
                                      name=f"wq{pfx}{nb}_{h}")
                    (nc.sync if h == 0 else nc.scalar).dma_start(
                        wq[:], wt5_d[nb, :, 16 * h:16 * (h + 1), :])
                    wq_f = wq[:].rearrange("p a b -> p (a b)")
                    mpos = mask_pool.tile([128, 2048], BF16, tag="masks",
                                          name=f"mp{pfx}{nb}_{h}")
                    nc.vector.tensor_scalar(
                        mpos[:], wq_f, thr_col[:], None, mybir.AluOpType.is_gt)
                    mneg = mask_pool.tile([128, 2048], BF16, tag="masks",
                                          name=f"mn{pfx}{nb}_{h}")
                    nc.vector.tensor_scalar(
                        mneg[:], wq_f, nthr_col[:], None, mybir.AluOpType.is_lt)
                    nc.vector.tensor_tensor(
                        qt[:, 16 * h:16 * (h + 1), :].rearrange("p a b -> p (a b)"),
                        mpos[:], mneg[:], mybir.AluOpType.subtract)

            # quant for nb=0 first so its DMA precedes the x stream
            qts = [qt_pool.tile([128, KO, 128], BF16, tag="qt", name=f"qt{nb}")
                   for nb in range(2)]
            emit_quant(0, qts[0])

            # ---- x -> bf16 resident [128ki, 512m] tiles, one per (mc, kb)
            # so each matmul depends only on its own k-block's cast, and the
            # accumulation chains pipeline right behind the x DMA stream.
            # Loads alternate between the two HWDGE rings (sync + scalar).
            xt = [[misc.tile([128, M_CHUNK], BF16, name=f"xt{mc}_{kb}")
                   for kb in range(KO)] for mc in range(N_MC)]
            for kb in range(KO):
                xf = xstage.tile([128, M_SH], F32, tag="xstage")
                dma_eng = nc.sync if kb % 2 == 0 else nc.scalar
                dma_eng.dma_start(xf[:], xt_d[128 * kb:128 * (kb + 1), :])
                for mc in range(N_MC):
                    src = xf[:, M_CHUNK * mc:M_CHUNK * (mc + 1)]
                    # split the f32->bf16 casts across ACT and DVE
                    if mc % 2 == 0:
                        nc.scalar.copy(xt[mc][kb][:], src)
                    else:
                        nc.vector.tensor_copy(xt[mc][kb][:], src)

            emit_quant(1, qts[1])

            def chain(nb, mc, qt):
                ps = psum_pool.tile([128, M_CHUNK], F32, tag="psum",
                                    name=f"ps{nb}_{mc}")
                for ko in range(KO):
                    nc.tensor.matmul(
                        ps[:], lhsT=qt[:, ko, :],
                        rhs=xt[mc][ko][:],
                        start=(ko == 0), stop=(ko == KO - 1))
                ob = out_pool.tile([128, M_CHUNK], F32, tag="outp",
                                   name=f"ob{nb}_{mc}")
                nc.scalar.activation(
                    ob[:], ps[:], mybir.ActivationFunctionType.Identity,
                    bias=bias_sb[:, nb:nb + 1], scale=s_col[:])
                nc.scalar.dma_start(
                    outT[128 * nb:128 * (nb + 1),
                         M_CHUNK * mc:M_CHUNK * (mc + 1)], ob[:])

            # ---- per n-block: matmul + fused evict (quant pipelined 2 ahead).
            # The first DEFER n-blocks only run m-chunks 0/1 (x still
            # streaming); their mc 2/3 chains run in a tail pass with a cheap
            # re-quantize, so no chain ever stalls holding a PSUM slot.
            DEFER = 0
            for nb in range(N_NB):
                qt = qts[nb]
                if nb + 2 < N_NB:
                    qts.append(qt_pool.tile([128, KO, 128], BF16, tag="qt",
                                            name=f"qt{nb + 2}"))
                    emit_quant(nb + 2, qts[nb + 2])
                mcs = (0, 1) if nb < DEFER else (0, 1, 2, 3)
                for mc in mcs:
                    chain(nb, mc, qt)
            for nb in range(DEFER):
                qt2 = qt_pool.tile([128, KO, 128], BF16, tag="qt",
                                   name=f"qt2_{nb}")
                emit_quant(nb, qt2, pfx="t")
                for mc in (2, 3):
                    chain(nb, mc, qt2)

    nc.compile()
    return nc


def kernel(x, weight, bias):
    global LAST_RESULTS
    x = np.asarray(x, dtype=np.float32)
    weight = np.ascontiguousarray(np.asarray(weight, dtype=np.float32))
    bias = np.ascontiguousarray(np.asarray(bias, dtype=np.float32))

    if "nc_scale" not in _CACHE:
        _CACHE["nc_scale"] = _build_scale()
        _CACHE["nc_main"] = _build_main()
    nc_scale, nc_main = _CACHE["nc_scale"], _CACHE["nc_main"]

    trace = bool(int(os.environ.get("KERNEL_TRACE", "0")))
    kw = {"trace": True, "trace_cores": [0]} if trace else {}

    # Launch A: distributed |W| partial sums (one distinct 1/8 slice each)
    in_a = [{"wredN": weight[WRED * c:WRED * (c + 1)]}
            for c in range(N_CORES)]
    res_a = run_bass_kernel_spmd(nc_scale, in_a, list(range(N_CORES)), **kw)
    partials = np.array(
        [res_a.results[c]["partial"][0, 0] for c in range(N_CORES)],
        dtype=np.float32)

    # Launch B: the matmul kernel
    xr = x.reshape(M_ALL, K)
    in_b = []
    for c in range(N_CORES):
        i, j = c // F_GRP, c % F_GRP
        w_sh = weight[N_SH * j:N_SH * (j + 1)]          # [2048 n, 4096 k]
        # wt5[nb, ki, kb, n] = w_sh[128*nb + n, 128*kb + ki]
        wt5 = np.ascontiguousarray(
            w_sh.reshape(N_NB, 128, KO, 128).transpose(0, 3, 2, 1))
        in_b.append({
            "xt_sh": np.ascontiguousarray(xr[M_SH * i:M_SH * (i + 1)].T),
            "wt5": wt5,
            "partials": partials,
            "bias_sh": bias[N_SH * j:N_SH * (j + 1)],
        })
    res_b = run_bass_kernel_spmd(nc_main, in_b, list(range(N_CORES)), **kw)
    LAST_RESULTS = (res_a, res_b)

    out = np.empty((M_ALL, N_OUT), dtype=np.float32)
    for c in range(N_CORES):
        i, j = c // F_GRP, c % F_GRP
        out[M_SH * i:M_SH * (i + 1), N_SH * j:N_SH * (j + 1)] = \
            res_b.results[c]["outT"].T
    return out.reshape(B, S, N_OUT)
